# revision 21
# baseline (speedup 1.0000x reference)
import os, sys

for _p in ("/opt/trn_rl_repo", "/root/.axon_site/_ro/trn_rl_repo"):
    if os.path.isdir(_p) and _p not in sys.path:
        sys.path.insert(0, _p)

import numpy as np
from contextlib import ExitStack

import concourse.bass as bass
import concourse.tile as tile
from concourse import bacc, mybir
from concourse.bass_utils import run_bass_kernel_spmd
from concourse._compat import with_exitstack

P = 128
L = 4096
D = 128
K = 256          # original_len (output bins)
KM1 = 255        # n_steps
SPC = 2          # samples per core
NCORES = 8
WIDTH = 0.125
WIN = 576        # window eval columns = 18*32
ND = 18          # number of shift matmuls
NLC = L // P     # 32 l-chunks
F32 = mybir.dt.float32
AF = mybir.ActivationFunctionType
ALU = mybir.AluOpType


def build_graph(nc):
    # ---- DRAM params (per-core shard) ----
    x_d = nc.dram_tensor("x", [SPC, L, D], F32, kind="ExternalInput").ap()
    w1_d = nc.dram_tensor("w1", [D, 2 * D], F32, kind="ExternalInput").ap()
    w2_d = nc.dram_tensor("w2", [2 * D, K], F32, kind="ExternalInput").ap()  # padded
    wr_d = nc.dram_tensor("wr", [D, 1], F32, kind="ExternalInput").ap()
    ident_d = nc.dram_tensor("ident", [P, P], F32, kind="ExternalInput").ap()
    tris_d = nc.dram_tensor("tris", [P, P], F32, kind="ExternalInput").ap()
    ones_d = nc.dram_tensor("onesm", [P, P], F32, kind="ExternalInput").ap()
    shift_d = nc.dram_tensor("shiftm", [P, P + ND], F32, kind="ExternalInput").ap()
    iota_d = nc.dram_tensor("iotaw", [P, WIN], F32, kind="ExternalInput").ap()
    kio_d = nc.dram_tensor("kiota", [P, K], F32, kind="ExternalInput").ap()
    qio_d = nc.dram_tensor("qiota", [P, P], F32, kind="ExternalInput").ap()
    msk_d = nc.dram_tensor("msk255", [P, 1], F32, kind="ExternalInput").ap()
    hlf_d = nc.dram_tensor("half255", [P, 1], F32, kind="ExternalInput").ap()
    lidx_d = nc.dram_tensor("lidx", [P, NLC], F32, kind="ExternalInput").ap()
    gs_d = nc.dram_tensor("gs_out", [SPC, L], F32, kind="ExternalOutput").ap()
    al_d = nc.dram_tensor("al_out", [SPC, L, K], F32, kind="ExternalOutput").ap()

    with tile.TileContext(nc) as tc:
        _body(tc, x_d, w1_d, w2_d, wr_d, ident_d, tris_d, ones_d, shift_d,
              iota_d, kio_d, qio_d, msk_d, hlf_d, lidx_d, gs_d, al_d)
    nc.compile()
    return nc


@with_exitstack
def _body(ctx, tc, x_d, w1_d, w2_d, wr_d, ident_d, tris_d, ones_d, shift_d,
          iota_d, kio_d, qio_d, msk_d, hlf_d, lidx_d, gs_d, al_d):
    nc = tc.nc

    def sb(name, shape):
        return nc.alloc_sbuf_tensor(name, list(shape), F32).ap()

    # ---- persistent SBUF ----
    w1s = sb("w1s", [P, 2 * D])
    w2s = sb("w2s", [P, 2, K])
    wrs = sb("wrs", [P, 1])
    ident = sb("identt", [P, P])
    tris = sb("triss", [P, P])
    onesm = sb("onesmm", [P, P])
    shiftm = sb("shiftmm", [P, P + ND])
    iotaw = sb("iotaww", [P, WIN])
    kiota = sb("kiotaa", [P, K])
    qiota = sb("qiotaa", [P, P])
    msk255 = sb("msk255s", [P, 1])
    hlf255 = sb("half255s", [P, 1])
    lidx = sb("lidxs", [P, NLC])
    xt = sb("xt", [P, SPC, L])          # X^T per sample  (d on partitions)
    tht = sb("tht", [P, 2, L])          # tanh(H)^T, hidden on partitions (2 tiles)
    r_sb = sb("r_sb", [P, NLC])         # r = X @ Wr in (p, chunk) layout
    rc_sb = sb("rc_sb", [P, NLC, 2])    # [ones | r] pairs per chunk

    # ---- pools ----
    xn_pool0 = ctx.enter_context(tc.tile_pool(name="xnat0", bufs=1))
    pb = ctx.enter_context(tc.tile_pool(name="pbig", bufs=2, space="PSUM"))
    pq = ctx.enter_context(tc.tile_pool(name="pq", bufs=1, space="PSUM"))
    psm = ctx.enter_context(tc.tile_pool(name="psm", bufs=2, space="PSUM"))
    xn_pool = ctx.enter_context(tc.tile_pool(name="xnat", bufs=1))
    et_pool = ctx.enter_context(tc.tile_pool(name="etp", bufs=2))
    win_pool = ctx.enter_context(tc.tile_pool(name="winp", bufs=1))
    smp = ctx.enter_context(tc.tile_pool(name="smsb", bufs=2))

    # X for sample 0 first so the PE pipeline starts ASAP; consts follow.
    xn0 = xn_pool0.tile([P, NLC, D], F32, tag="xn0")
    xv0 = x_d[0].rearrange("(p c) d -> p c d", p=P)
    for g in range(4):
        nc.sync.dma_start(out=xn0[:, 8 * g:8 * (g + 1), :],
                          in_=xv0[:, 8 * g:8 * (g + 1), :])
    nc.sync.dma_start(out=ident, in_=ident_d)
    nc.sync.dma_start(out=w1s, in_=w1_d)
    nc.sync.dma_start(out=w2s, in_=w2_d.rearrange("(t p) k -> p t k", p=P))
    for dst, src in ((wrs, wr_d), (tris, tris_d),
                     (onesm, ones_d), (shiftm, shift_d), (iotaw, iota_d),
                     (kiota, kio_d), (qiota, qio_d), (msk255, msk_d),
                     (hlf255, hlf_d), (lidx, lidx_d)):
        nc.sync.dma_start(out=dst, in_=src)

    for s in range(SPC):
        # ================= load X and transpose (l = 32*p + c permuted order) ====
        if s == 0:
            xn = xn0
        else:
            xn = xn_pool.tile([P, NLC, D], F32, tag="xn")
            xview = x_d[s].rearrange("(p c) d -> p c d", p=P)
            for g in range(4):
                nc.sync.dma_start(out=xn[:, 8 * g:8 * (g + 1), :],
                                  in_=xview[:, 8 * g:8 * (g + 1), :])
        for g in range(4):  # 8 transposes per psum group
            pt = pb.tile([P, 1024], F32, space="PSUM", tag="big")
            for u in range(8):
                c = g * 8 + u
                nc.tensor.matmul(pt[:, u * 128:(u + 1) * 128], lhsT=xn[:, c, :],
                                 rhs=ident[:], is_transpose=True,
                                 start=(u % 4 == 0), stop=(u % 4 == 3))
            if g % 2 == 0:
                nc.vector.tensor_copy(xt[:, s, g * 1024:(g + 1) * 1024], pt[:])
            else:
                nc.scalar.activation(xt[:, s, g * 1024:(g + 1) * 1024], pt[:], AF.Copy)

        # ================= r = X @ Wr  -> [p, chunk] =============================
        pr = psm.tile([P, 512], F32, space="PSUM", tag="sm")
        for c in range(NLC):
            nc.tensor.matmul(pr[:, c:c + 1], lhsT=xt[:, s, c * 128:(c + 1) * 128],
                             rhs=wrs[:], start=(c == 0), stop=(c == NLC - 1))
        nc.vector.tensor_copy(r_sb[:], pr[:, 0:NLC])
        nc.vector.memset(rc_sb[:], 1.0)
        nc.vector.tensor_copy(rc_sb[:, :, 1], r_sb[:])

        # ================= mm1 + tanh: tht = tanh(W1^T @ X^T) ====================
        for mt in range(2):
            for g in range(4):
                ph = pb.tile([P, 1024], F32, space="PSUM", tag="big")
                for u in range(2):
                    sl = slice(g * 1024 + u * 512, g * 1024 + (u + 1) * 512)
                    nc.tensor.matmul(ph[:, u * 512:(u + 1) * 512],
                                     lhsT=w1s[:, mt * 128:(mt + 1) * 128],
                                     rhs=xt[:, s, sl], start=True, stop=True)
                nc.scalar.activation(tht[:, mt, g * 1024:(g + 1) * 1024], ph[:], AF.Tanh)

        # ========== mm2 + exp (streamed) + softmax-sum matmuls ===================
        pmu = psm.tile([P, 512], F32, space="PSUM", tag="sm")
        for g in range(8):
            pa = pb.tile([P, 1024], F32, space="PSUM", tag="big")
            for u in range(4):
                lc = g * 4 + u
                for ct in range(2):
                    nc.tensor.matmul(pa[:, u * 256:(u + 1) * 256],
                                     lhsT=tht[:, ct, lc * 128:(lc + 1) * 128],
                                     rhs=w2s[:, ct, :],
                                     start=(ct == 0 and u % 2 == 0),
                                     stop=(ct == 1 and u % 2 == 1))
            etg = et_pool.tile([P, 4, K], F32, tag="et")
            nc.scalar.activation(etg[:], pa[:], AF.Exp)
            for u in range(4):
                lc = g * 4 + u
                nc.tensor.matmul(pmu[0:2, 0:256],
                                 lhsT=rc_sb[:, lc, :],
                                 rhs=etg[:, u, :],
                                 start=(lc == 0), stop=(lc == NLC - 1))

        # ================= modes =================================================
        mu2 = smp.tile([2, 256], F32, tag="mu2")
        nc.vector.tensor_copy(mu2[:], pmu[0:2, 0:256])
        pmt = psm.tile([P, 512], F32, space="PSUM", tag="sm")
        for mt in range(2):
            nc.tensor.matmul(pmt[:, 2 * mt:2 * mt + 2],
                             lhsT=mu2[:, mt * 128:(mt + 1) * 128],
                             rhs=ident[0:2, 0:2], is_transpose=True,
                             start=(mt == 0), stop=(mt == 1))
        musb = smp.tile([P, 2, 2], F32, tag="p4")
        nc.vector.tensor_copy(musb[:], pmt[:, 0:4])
        rden = smp.tile([P, 2], F32, tag="p2a")
        nc.vector.reciprocal(rden[:], musb[:, :, 0])
        mmu = smp.tile([P, 2], F32, tag="p2b")
        nc.vector.tensor_tensor(out=mmu[:], in0=musb[:, :, 1], in1=rden[:], op=ALU.mult)
        expmu = smp.tile([P, 2], F32, tag="p2c")
        nc.scalar.activation(expmu[:], mmu[:], AF.Exp)
        nc.vector.tensor_tensor(out=expmu[:, 1:2], in0=expmu[:, 1:2],
                                in1=msk255[:], op=ALU.mult)

        # modes = clip(cumsum(softmax([0, mu]))[:255]) ; k = mt*128 + p
        pmc = psm.tile([P, 512], F32, space="PSUM", tag="sm")
        nc.tensor.matmul(pmc[:, 0:1], lhsT=tris[:], rhs=expmu[:, 0:1], start=True, stop=False)
        nc.tensor.matmul(pmc[:, 1:2], lhsT=tris[:], rhs=expmu[:, 1:2], start=False, stop=False)
        nc.tensor.matmul(pmc[:, 1:2], lhsT=onesm[:], rhs=expmu[:, 0:1], start=False, stop=False)
        nc.tensor.matmul(pmc[:, 2:3], lhsT=onesm[:], rhs=expmu[:, 0:1], start=False, stop=False)
        nc.tensor.matmul(pmc[:, 2:3], lhsT=onesm[:], rhs=expmu[:, 1:2], start=False, stop=True)
        mc = smp.tile([P, 3], F32, tag="p3")
        nc.vector.tensor_copy(mc[:], pmc[:, 0:3])
        modes = smp.tile([P, 2], F32, tag="modes")
        rz = smp.tile([P, 1], F32, tag="p1a")
        nc.vector.tensor_scalar(out=rz[:], in0=mc[:, 2:3], scalar1=1.0, scalar2=None, op0=ALU.add)
        nc.vector.reciprocal(rz[:], rz[:])
        nc.vector.tensor_scalar(out=modes[:], in0=mc[:, 0:2], scalar1=1.0, scalar2=None, op0=ALU.add)
        nc.vector.tensor_scalar(out=modes[:], in0=modes[:], scalar1=rz[:, 0:1], scalar2=None, op0=ALU.mult)
        nc.vector.tensor_scalar(out=modes[:], in0=modes[:], scalar1=1e-4, scalar2=0.9999,
                                op0=ALU.max, op1=ALU.min)
        nc.vector.tensor_tensor(out=modes[:, 1:2], in0=modes[:, 1:2],
                                in1=msk255[:], op=ALU.mult)
        nc.vector.tensor_tensor(out=modes[:, 1:2], in0=modes[:, 1:2],
                                in1=hlf255[:], op=ALU.add)

        # ================= per-mode params (all [P, 2]) ==========================
        def ptile(tag):
            return smp.tile([P, 2], F32, tag=tag, name="prm_" + tag)

        av = ptile("av")
        nc.vector.tensor_scalar(out=av[:], in0=modes[:], scalar1=WIDTH / 2, scalar2=0.0,
                                op0=ALU.subtract, op1=ALU.max)
        nc.vector.tensor_scalar(out=av[:], in0=av[:], scalar1=1.0 - WIDTH, scalar2=None, op0=ALU.min)
        bv = ptile("bv")
        nc.vector.tensor_scalar(out=bv[:], in0=av[:], scalar1=WIDTH, scalar2=None, op0=ALU.add)
        d1 = ptile("d1")
        nc.vector.tensor_tensor(out=d1[:], in0=modes[:], in1=av[:], op=ALU.subtract)
        d2 = ptile("d2")
        nc.vector.tensor_tensor(out=d2[:], in0=bv[:], in1=modes[:], op=ALU.subtract)
        s1 = ptile("s1")
        nc.vector.reciprocal(s1[:], d1[:])
        t1 = ptile("t1")
        nc.vector.reciprocal(t1[:], d2[:])
        clw = ptile("clw")
        nc.vector.tensor_scalar(out=clw[:], in0=d1[:], scalar1=8.0, scalar2=None, op0=ALU.mult)
        crn = ptile("crn")
        nc.vector.tensor_scalar(out=crn[:], in0=d2[:], scalar1=-8.0, scalar2=None, op0=ALU.mult)
        nc.vector.tensor_tensor(out=clw[:, 1:2], in0=clw[:, 1:2], in1=msk255[:], op=ALU.mult)
        nc.vector.tensor_tensor(out=crn[:, 1:2], in0=crn[:, 1:2], in1=msk255[:], op=ALU.mult)
        l0 = ptile("l0")
        nc.vector.tensor_scalar(out=l0[:], in0=av[:], scalar1=float(L - 1), scalar2=None, op0=ALU.mult)
        l0i = smp.tile([P, 2], mybir.dt.int32, tag="l0i", name="l0i")
        nc.vector.tensor_copy(l0i[:], l0[:])
        nc.vector.tensor_scalar(out=l0i[:], in0=l0i[:], scalar1=-32, scalar2=None,
                                op0=ALU.bitwise_and)
        l0q = ptile("l0q")
        nc.vector.tensor_copy(l0q[:], l0i[:])
        qk = ptile("qk")
        nc.vector.tensor_scalar(out=qk[:], in0=l0q[:], scalar1=1.0 / 32.0, scalar2=None, op0=ALU.mult)
        uu = ptile("uu")
        nc.vector.tensor_scalar(out=uu[:], in0=l0q[:], scalar1=1.0 / (L - 1), scalar2=None, op0=ALU.mult)
        bAL = ptile("bAL")
        nc.vector.tensor_tensor(out=bAL[:], in0=uu[:], in1=av[:], op=ALU.subtract)
        nc.vector.tensor_tensor(out=bAL[:], in0=bAL[:], in1=s1[:], op=ALU.mult)
        sAL = ptile("sAL")
        nc.vector.tensor_scalar(out=sAL[:], in0=s1[:], scalar1=1.0 / (L - 1), scalar2=None, op0=ALU.mult)
        bAR = ptile("bAR")
        nc.vector.tensor_tensor(out=bAR[:], in0=bv[:], in1=uu[:], op=ALU.subtract)
        nc.vector.tensor_tensor(out=bAR[:], in0=bAR[:], in1=t1[:], op=ALU.mult)
        sAR = ptile("sAR")
        nc.vector.tensor_scalar(out=sAR[:], in0=t1[:], scalar1=-1.0 / (L - 1), scalar2=None, op0=ALU.mult)

        # ================= window eval + selection matmuls =======================
        pqt = pq.tile([P, 1024], F32, space="PSUM", tag="q")
        mm_i = 0
        for mt in range(2):
            sel = win_pool.tile([P, P], F32, tag="sel", bufs=2)
            nc.vector.tensor_tensor(out=sel[:], in0=qiota[:],
                                    in1=qk[:, mt:mt + 1].to_broadcast([P, P]),
                                    op=ALU.is_equal)
            for side in range(2):  # 0 = L, 1 = R
                sA = sAL if side == 0 else sAR
                bA = bAL if side == 0 else bAR
                wt = win_pool.tile([P, WIN], F32, tag="wt", bufs=2)
                nc.scalar.activation(wt[:], iotaw[:], AF.Relu,
                                     bias=bA[:, mt:mt + 1], scale=sA[:, mt:mt + 1])
                nc.scalar.activation(wt[:], wt[:], AF.Square)
                w4t = win_pool.tile([P, WIN], F32, tag="w4t", bufs=2)
                nc.vector.tensor_tensor(out=w4t[:], in0=wt[:], in1=wt[:], op=ALU.mult)
                nc.vector.tensor_tensor(out=w4t[:], in0=w4t[:], in1=w4t[:], op=ALU.mult)
                nc.vector.tensor_scalar(out=w4t[:], in0=w4t[:], scalar1=1.0, scalar2=None, op0=ALU.min)
                w16 = win_pool.tile([P, WIN], F32, tag="w16", bufs=2)
                nc.vector.tensor_tensor(out=w16[:], in0=w4t[:], in1=w4t[:], op=ALU.mult)
                dd = win_pool.tile([P, WIN], F32, tag="dd", bufs=2)
                if side == 0:
                    nc.vector.tensor_copy(dd[:, 0:1], w16[:, 0:1])
                else:
                    nc.vector.tensor_scalar(out=dd[:, 0:1], in0=w16[:, 0:1],
                                            scalar1=1.0, scalar2=None, op0=ALU.subtract)
                nc.vector.tensor_tensor(out=dd[:, 1:WIN], in0=w16[:, 1:WIN],
                                        in1=w16[:, 0:WIN - 1], op=ALU.subtract)
                selw = win_pool.tile([P, P], F32, tag="selw", bufs=2)
                wvec = clw if side == 0 else crn
                nc.vector.tensor_scalar(out=selw[:], in0=sel[:], scalar1=wvec[:, mt:mt + 1],
                                        scalar2=None, op0=ALU.mult)
                nc.tensor.matmul(pqt[:, 0:512], lhsT=selw[:], rhs=dd[:, 0:512],
                                 start=(mm_i == 0), stop=(mm_i == 3))
                nc.tensor.matmul(pqt[:, 512:WIN], lhsT=selw[:], rhs=dd[:, 512:WIN],
                                 start=(mm_i == 0), stop=(mm_i == 3))
                mm_i += 1
        qsb = win_pool.tile([P, WIN], F32, tag="qsb", bufs=2)
        nc.vector.tensor_copy(qsb[:], pqt[:, 0:WIN])

        # ================= shift-realign + cumsum ================================
        pgb = psm.tile([P, 512], F32, space="PSUM", tag="sm")
        for d in range(ND):
            nc.tensor.matmul(pgb[:, 0:32], lhsT=shiftm[:, ND - d:ND - d + P],
                             rhs=qsb[:, 32 * d:32 * d + 32],
                             start=(d == 0), stop=(d == ND - 1))
        cur = smp.tile([P, 32], F32, tag="cs0")
        nc.vector.tensor_copy(cur[:], pgb[:, 0:32])
        for si, sft in enumerate((1, 2, 4, 8, 16)):
            nxt = smp.tile([P, 32], F32, tag=f"cs{si + 1}", name=f"csn{si}")
            nc.vector.tensor_copy(nxt[:, 0:sft], cur[:, 0:sft])
            nc.vector.tensor_tensor(out=nxt[:, sft:32], in0=cur[:, sft:32],
                                    in1=cur[:, 0:32 - sft], op=ALU.add)
            cur = nxt
        ppf = psm.tile([P, 512], F32, space="PSUM", tag="sm")
        nc.tensor.matmul(ppf[:, 0:1], lhsT=tris[:], rhs=cur[:, 31:32], start=True, stop=True)
        pfx = smp.tile([P, 1], F32, tag="pfx")
        nc.vector.tensor_copy(pfx[:], ppf[:, 0:1])
        gs_t = smp.tile([P, 32], F32, tag="gst")
        nc.vector.tensor_scalar(out=gs_t[:], in0=cur[:], scalar1=pfx[:, 0:1],
                                scalar2=None, op0=ALU.add)
        nc.sync.dma_start(out=gs_d[s].rearrange("(p j) -> p j", p=P), in_=gs_t[:])

        # ======== almat: gpsimd local_scatter of (1-f, f) pairs + cast DMA ======
        kci = smp.tile([P, 32], mybir.dt.int32, tag="kci", name="kci")
        nc.vector.tensor_copy(kci[:], gs_t[:])
        kcf = smp.tile([P, 32], F32, tag="kcf")
        nc.vector.tensor_copy(kcf[:], kci[:])
        gtt = smp.tile([P, 32], F32, tag="gtt")
        nc.vector.tensor_tensor(out=gtt[:], in0=kcf[:], in1=gs_t[:], op=ALU.is_gt)
        k0f = smp.tile([P, 32], F32, tag="k0f")
        nc.vector.tensor_tensor(out=k0f[:], in0=kcf[:], in1=gtt[:], op=ALU.subtract)
        nc.vector.tensor_scalar(out=k0f[:], in0=k0f[:], scalar1=float(K - 2),
                                scalar2=None, op0=ALU.min)
        fr2 = smp.tile([P, 32], F32, tag="fr2")
        nc.vector.tensor_tensor(out=fr2[:], in0=gs_t[:], in1=k0f[:], op=ALU.subtract)
        pairs = smp.tile([P, 32, 2], F32, tag="pairs")
        nc.vector.tensor_copy(pairs[:, :, 1], fr2[:])
        nc.vector.tensor_scalar(out=pairs[:, :, 0], in0=fr2[:], scalar1=-1.0,
                                scalar2=1.0, op0=ALU.mult, op1=ALU.add)
        d16 = smp.tile([P, 32, 2], mybir.dt.float16, tag="d16", name="d16")
        nc.vector.tensor_copy(d16[:], pairs[:])
        idxf = smp.tile([P, 32, 2], F32, tag="idxf")
        nc.vector.tensor_tensor(out=idxf[:, :, 0], in0=k0f[:], in1=lidx[:], op=ALU.add)
        nc.vector.tensor_scalar(out=idxf[:, :, 1], in0=idxf[:, :, 0], scalar1=1.0,
                                scalar2=None, op0=ALU.add)
        i16 = smp.tile([P, 32, 2], mybir.dt.int16, tag="i16", name="i16")
        nc.vector.tensor_copy(i16[:], idxf[:])
        alv = al_d[s].rearrange("(p j) k -> p (j k)", p=P)
        for e in range(8):
            dst16 = win_pool.tile([P, 1024], mybir.dt.float16, tag="dst16", bufs=2)
            nc.gpsimd.local_scatter(out_ap=dst16[:], data_ap=d16[:, 4 * e:4 * e + 4, :],
                                    idxs_ap=i16[:, 4 * e:4 * e + 4, :],
                                    channels=P, num_elems=1024, num_idxs=8)
            dst32 = win_pool.tile([P, 1024], F32, tag="dst32", bufs=3)
            if s == 0:
                nc.gpsimd.tensor_copy(dst32[:], dst16[:])
            else:
                nc.vector.tensor_copy(dst32[:], dst16[:])
            nc.sync.dma_start(out=alv[:, 1024 * e:1024 * (e + 1)], in_=dst32[:])


# ================================ host side ==================================

def _consts():
    c = {}
    c["ident"] = np.eye(P, dtype=np.float32)
    c["tris"] = np.triu(np.ones((P, P), dtype=np.float32), 1)
    c["onesm"] = np.ones((P, P), dtype=np.float32)
    c["shiftm"] = np.eye(P, P + ND, k=ND, dtype=np.float32)
    c["iotaw"] = np.tile(np.arange(WIN, dtype=np.float32), (P, 1))
    c["kiota"] = np.tile(np.arange(K, dtype=np.float32), (P, 1))
    c["qiota"] = np.tile(np.arange(P, dtype=np.float32), (P, 1))
    m = np.ones((P, 1), dtype=np.float32); m[P - 1, 0] = 0.0
    c["msk255"] = m
    h = np.zeros((P, 1), dtype=np.float32); h[P - 1, 0] = 0.5
    c["half255"] = h
    li = np.tile((np.arange(NLC) % 4) * K, (P, 1))
    c["lidx"] = li.astype(np.float32)
    return c


_COMPILED = None
LAST_EXEC_NS = None


def _get_compiled():
    global _COMPILED
    if _COMPILED is None:
        nc = bacc.Bacc("TRN2", target_bir_lowering=False, debug=False,
                       num_devices=NCORES)
        build_graph(nc)
        _COMPILED = nc
    return _COMPILED


def kernel(input_seq, mask, W1, b1, W2, Wr, br, _run_kwargs=None):
    input_seq = np.asarray(input_seq, dtype=np.float32)
    W1 = np.ascontiguousarray(np.asarray(W1, dtype=np.float32))
    W2 = np.asarray(W2, dtype=np.float32)
    Wr = np.ascontiguousarray(np.asarray(Wr, dtype=np.float32))
    B = input_seq.shape[0]
    w2p = np.ascontiguousarray(
        np.concatenate([W2, np.zeros((2 * D, 1), np.float32)], axis=1))
    consts = _consts()
    nc = _get_compiled()
    in_maps = []
    for cix in range(NCORES):
        m = dict(consts)
        m["x"] = np.ascontiguousarray(input_seq[cix * SPC:(cix + 1) * SPC])
        m["w1"] = W1
        m["w2"] = w2p
        m["wr"] = Wr
        in_maps.append(m)
    res = run_bass_kernel_spmd(nc, in_maps, core_ids=list(range(NCORES)),
                               **(_run_kwargs or {}))
    global LAST_EXEC_NS
    if getattr(res, "exec_time_ns", None):
        LAST_EXEC_NS = res.exec_time_ns
    gs = np.concatenate([res.results[i]["gs_out"] for i in range(NCORES)], axis=0)
    al = np.concatenate([res.results[i]["al_out"] for i in range(NCORES)], axis=0)
    return gs.reshape(B, L).astype(np.float32), al.reshape(B, L, K).astype(np.float32)


# revision 24
# speedup vs baseline: 1.5312x; 1.5312x over previous
import os, sys

for _p in ("/opt/trn_rl_repo", "/root/.axon_site/_ro/trn_rl_repo"):
    if os.path.isdir(_p) and _p not in sys.path:
        sys.path.insert(0, _p)

import numpy as np
from contextlib import ExitStack

import concourse.bass as bass
import concourse.tile as tile
from concourse import bacc, mybir
from concourse.bass_utils import run_bass_kernel_spmd
from concourse._compat import with_exitstack

P = 128
L = 4096
D = 128
K = 256          # original_len (output bins)
KM1 = 255        # n_steps
SPC = 2          # samples per core
NCORES = 8
WIDTH = 0.125
WIN = 576        # window eval columns = 18*32
ND = 18          # number of shift matmuls
NLC = L // P     # 32 l-chunks
F32 = mybir.dt.float32
AF = mybir.ActivationFunctionType
ALU = mybir.AluOpType


def build_graph(nc):
    # ---- DRAM params (per-core shard) ----
    x_d = nc.dram_tensor("x", [SPC, L, D], F32, kind="ExternalInput").ap()
    w1_d = nc.dram_tensor("w1", [D, 2 * D], F32, kind="ExternalInput").ap()
    w2_d = nc.dram_tensor("w2", [2 * D, K], F32, kind="ExternalInput").ap()  # padded
    wr_d = nc.dram_tensor("wr", [D, 1], F32, kind="ExternalInput").ap()
    ident_d = nc.dram_tensor("ident", [P, P], F32, kind="ExternalInput").ap()
    tris_d = nc.dram_tensor("tris", [P, P], F32, kind="ExternalInput").ap()
    ones_d = nc.dram_tensor("onesm", [P, P], F32, kind="ExternalInput").ap()
    shift_d = nc.dram_tensor("shiftm", [P, P + ND], F32, kind="ExternalInput").ap()
    iota_d = nc.dram_tensor("iotaw", [P, WIN], F32, kind="ExternalInput").ap()
    kio_d = nc.dram_tensor("kiota", [P, K], F32, kind="ExternalInput").ap()
    qio_d = nc.dram_tensor("qiota", [P, P], F32, kind="ExternalInput").ap()
    msk_d = nc.dram_tensor("msk255", [P, 1], F32, kind="ExternalInput").ap()
    hlf_d = nc.dram_tensor("half255", [P, 1], F32, kind="ExternalInput").ap()
    lidx_d = nc.dram_tensor("lidx", [P, NLC], F32, kind="ExternalInput").ap()
    gs_d = nc.dram_tensor("gs_out", [SPC, L], F32, kind="ExternalOutput").ap()
    al_d = nc.dram_tensor("al_out", [SPC, L, K], F32, kind="ExternalOutput").ap()

    with tile.TileContext(nc) as tc:
        _body(tc, x_d, w1_d, w2_d, wr_d, ident_d, tris_d, ones_d, shift_d,
              iota_d, kio_d, qio_d, msk_d, hlf_d, lidx_d, gs_d, al_d)
    nc.compile()
    return nc


@with_exitstack
def _body(ctx, tc, x_d, w1_d, w2_d, wr_d, ident_d, tris_d, ones_d, shift_d,
          iota_d, kio_d, qio_d, msk_d, hlf_d, lidx_d, gs_d, al_d):
    nc = tc.nc

    def sb(name, shape):
        return nc.alloc_sbuf_tensor(name, list(shape), F32).ap()

    # ---- persistent SBUF ----
    w1s = sb("w1s", [P, 2 * D])
    w2s = sb("w2s", [P, 2, K])
    wrs = sb("wrs", [P, 1])
    ident = sb("identt", [P, P])
    tris = sb("triss", [P, P])
    onesm = sb("onesmm", [P, P])
    shiftm = sb("shiftmm", [P, P + ND])
    iotaw = sb("iotaww", [P, WIN])
    kiota = sb("kiotaa", [P, K])
    qiota = sb("qiotaa", [P, P])
    msk255 = sb("msk255s", [P, 1])
    hlf255 = sb("half255s", [P, 1])
    lidx = sb("lidxs", [P, NLC])
    def sbr(name, shape):
        return nc.alloc_sbuf_tensor(name, list(shape), mybir.dt.float32r).ap()

    xt = sbr("xt", [P, SPC, L])         # X^T per sample (d on partitions), fp32r
    tht = sbr("tht", [P, 2, L])         # tanh(H)^T, fp32r
    w1r = sbr("w1r", [P, 2 * D])        # fp32r-rounded W1
    w2r = sbr("w2r", [P, 2, K])         # fp32r-rounded W2
    r_sb = sb("r_sb", [P, NLC])         # r = X @ Wr in (p, chunk) layout
    rc_sb = sbr("rc_sb", [P, NLC, 2])   # [ones | r] pairs per chunk, fp32r

    # ---- pools ----
    xn_pool0 = ctx.enter_context(tc.tile_pool(name="xnat0", bufs=1))
    pb = ctx.enter_context(tc.tile_pool(name="pbig", bufs=2, space="PSUM"))
    pq = ctx.enter_context(tc.tile_pool(name="pq", bufs=1, space="PSUM"))
    psm = ctx.enter_context(tc.tile_pool(name="psm", bufs=2, space="PSUM"))
    xn_pool = ctx.enter_context(tc.tile_pool(name="xnat", bufs=1))
    et_pool = ctx.enter_context(tc.tile_pool(name="etp", bufs=2))
    win_pool = ctx.enter_context(tc.tile_pool(name="winp", bufs=1))
    smp = ctx.enter_context(tc.tile_pool(name="smsb", bufs=2))

    # X for sample 0 first so the PE pipeline starts ASAP; consts follow.
    xn0 = xn_pool0.tile([P, NLC, D], F32, tag="xn0")
    xv0 = x_d[0].rearrange("(p c) d -> p c d", p=P)
    for g in range(4):
        nc.sync.dma_start(out=xn0[:, 8 * g:8 * (g + 1), :],
                          in_=xv0[:, 8 * g:8 * (g + 1), :])
    nc.sync.dma_start(out=ident, in_=ident_d)
    nc.sync.dma_start(out=w1s, in_=w1_d)
    nc.sync.dma_start(out=w2s, in_=w2_d.rearrange("(t p) k -> p t k", p=P))
    nc.vector.tensor_copy(w1r[:], w1s[:])
    nc.vector.tensor_copy(w2r[:], w2s[:])
    for dst, src in ((wrs, wr_d), (tris, tris_d),
                     (onesm, ones_d), (shiftm, shift_d), (iotaw, iota_d),
                     (kiota, kio_d), (qiota, qio_d), (msk255, msk_d),
                     (hlf255, hlf_d), (lidx, lidx_d)):
        nc.sync.dma_start(out=dst, in_=src)

    for s in range(SPC):
        # ================= load X and transpose (l = 32*p + c permuted order) ====
        if s == 0:
            xn = xn0
        else:
            xn = xn_pool.tile([P, NLC, D], F32, tag="xn")
            xview = x_d[s].rearrange("(p c) d -> p c d", p=P)
            for g in range(4):
                nc.sync.dma_start(out=xn[:, 8 * g:8 * (g + 1), :],
                                  in_=xview[:, 8 * g:8 * (g + 1), :])
        for g in range(4):  # 8 transposes per psum group
            pt = pb.tile([P, 1024], F32, space="PSUM", tag="big")
            for u in range(8):
                c = g * 8 + u
                nc.tensor.matmul(pt[:, u * 128:(u + 1) * 128], lhsT=xn[:, c, :],
                                 rhs=ident[:], is_transpose=True,
                                 start=(u % 4 == 0), stop=(u % 4 == 3))
            if g % 2 == 0:
                nc.vector.tensor_copy(xt[:, s, g * 1024:(g + 1) * 1024], pt[:])
            else:
                nc.scalar.activation(xt[:, s, g * 1024:(g + 1) * 1024], pt[:], AF.Copy)

        # ================= r = X @ Wr  -> [p, chunk] =============================
        pr = psm.tile([P, 512], F32, space="PSUM", tag="sm")
        for c in range(NLC):
            nc.tensor.matmul(pr[:, c:c + 1],
                             lhsT=xt[:, s, c * 128:(c + 1) * 128].bitcast(F32),
                             rhs=wrs[:], start=(c == 0), stop=(c == NLC - 1))
        nc.vector.tensor_copy(r_sb[:], pr[:, 0:NLC])
        rcf = smp.tile([P, NLC, 2], F32, tag="rcf")
        nc.vector.memset(rcf[:], 1.0)
        nc.vector.tensor_copy(rcf[:, :, 1], r_sb[:])
        nc.vector.tensor_copy(rc_sb[:], rcf[:])

        # ================= mm1 + tanh: tht = tanh(W1^T @ X^T) ====================
        for mt in range(2):
            for g in range(4):
                ph = pb.tile([P, 1024], F32, space="PSUM", tag="big")
                for u in range(2):
                    sl = slice(g * 1024 + u * 512, g * 1024 + (u + 1) * 512)
                    nc.tensor.matmul(ph[:, u * 512:(u + 1) * 512],
                                     lhsT=w1r[:, mt * 128:(mt + 1) * 128],
                                     rhs=xt[:, s, sl],
                                     start=True, stop=True)
                nc.scalar.activation(tht[:, mt, g * 1024:(g + 1) * 1024], ph[:], AF.Tanh)

        # ========== mm2 + exp (streamed) + softmax-sum matmuls ===================
        pmu = psm.tile([P, 512], F32, space="PSUM", tag="sm")
        for g in range(8):
            pa = pb.tile([P, 1024], F32, space="PSUM", tag="big")
            for u in range(4):
                lc = g * 4 + u
                for ct in range(2):
                    nc.tensor.matmul(pa[:, u * 256:(u + 1) * 256],
                                     lhsT=tht[:, ct, lc * 128:(lc + 1) * 128],
                                     rhs=w2r[:, ct, :],
                                     start=(ct == 0 and u % 2 == 0),
                                     stop=(ct == 1 and u % 2 == 1))
            etg = et_pool.tile([P, 4, K], mybir.dt.float32r, tag="et")
            nc.scalar.activation(etg[:], pa[:], AF.Exp)
            for u in range(4):
                lc = g * 4 + u
                nc.tensor.matmul(pmu[0:2, 0:256],
                                 lhsT=rc_sb[:, lc, :],
                                 rhs=etg[:, u, :],
                                 start=(lc == 0), stop=(lc == NLC - 1))

        # ================= modes =================================================
        mu2 = smp.tile([2, 256], F32, tag="mu2")
        nc.vector.tensor_copy(mu2[:], pmu[0:2, 0:256])
        pmt = psm.tile([P, 512], F32, space="PSUM", tag="sm")
        for mt in range(2):
            nc.tensor.matmul(pmt[:, 2 * mt:2 * mt + 2],
                             lhsT=mu2[:, mt * 128:(mt + 1) * 128],
                             rhs=ident[0:2, 0:2], is_transpose=True,
                             start=(mt == 0), stop=(mt == 1))
        musb = smp.tile([P, 2, 2], F32, tag="p4")
        nc.vector.tensor_copy(musb[:], pmt[:, 0:4])
        rden = smp.tile([P, 2], F32, tag="p2a")
        nc.vector.reciprocal(rden[:], musb[:, :, 0])
        mmu = smp.tile([P, 2], F32, tag="p2b")
        nc.vector.tensor_tensor(out=mmu[:], in0=musb[:, :, 1], in1=rden[:], op=ALU.mult)
        expmu = smp.tile([P, 2], F32, tag="p2c")
        nc.scalar.activation(expmu[:], mmu[:], AF.Exp)
        nc.vector.tensor_tensor(out=expmu[:, 1:2], in0=expmu[:, 1:2],
                                in1=msk255[:], op=ALU.mult)

        # modes = clip(cumsum(softmax([0, mu]))[:255]) ; k = mt*128 + p
        pmc = psm.tile([P, 512], F32, space="PSUM", tag="sm")
        nc.tensor.matmul(pmc[:, 0:1], lhsT=tris[:], rhs=expmu[:, 0:1], start=True, stop=False)
        nc.tensor.matmul(pmc[:, 1:2], lhsT=tris[:], rhs=expmu[:, 1:2], start=False, stop=False)
        nc.tensor.matmul(pmc[:, 1:2], lhsT=onesm[:], rhs=expmu[:, 0:1], start=False, stop=False)
        nc.tensor.matmul(pmc[:, 2:3], lhsT=onesm[:], rhs=expmu[:, 0:1], start=False, stop=False)
        nc.tensor.matmul(pmc[:, 2:3], lhsT=onesm[:], rhs=expmu[:, 1:2], start=False, stop=True)
        mc = smp.tile([P, 3], F32, tag="p3")
        nc.vector.tensor_copy(mc[:], pmc[:, 0:3])
        modes = smp.tile([P, 2], F32, tag="modes")
        rz = smp.tile([P, 1], F32, tag="p1a")
        nc.vector.tensor_scalar(out=rz[:], in0=mc[:, 2:3], scalar1=1.0, scalar2=None, op0=ALU.add)
        nc.vector.reciprocal(rz[:], rz[:])
        nc.vector.tensor_scalar(out=modes[:], in0=mc[:, 0:2], scalar1=1.0, scalar2=None, op0=ALU.add)
        nc.vector.tensor_scalar(out=modes[:], in0=modes[:], scalar1=rz[:, 0:1], scalar2=None, op0=ALU.mult)
        nc.vector.tensor_scalar(out=modes[:], in0=modes[:], scalar1=1e-4, scalar2=0.9999,
                                op0=ALU.max, op1=ALU.min)
        nc.vector.tensor_tensor(out=modes[:, 1:2], in0=modes[:, 1:2],
                                in1=msk255[:], op=ALU.mult)
        nc.vector.tensor_tensor(out=modes[:, 1:2], in0=modes[:, 1:2],
                                in1=hlf255[:], op=ALU.add)

        # ================= per-mode params (all [P, 2]) ==========================
        def ptile(tag):
            return smp.tile([P, 2], F32, tag=tag, name="prm_" + tag)

        av = ptile("av")
        nc.vector.tensor_scalar(out=av[:], in0=modes[:], scalar1=WIDTH / 2, scalar2=0.0,
                                op0=ALU.subtract, op1=ALU.max)
        nc.vector.tensor_scalar(out=av[:], in0=av[:], scalar1=1.0 - WIDTH, scalar2=None, op0=ALU.min)
        bv = ptile("bv")
        nc.vector.tensor_scalar(out=bv[:], in0=av[:], scalar1=WIDTH, scalar2=None, op0=ALU.add)
        d1 = ptile("d1")
        nc.vector.tensor_tensor(out=d1[:], in0=modes[:], in1=av[:], op=ALU.subtract)
        d2 = ptile("d2")
        nc.vector.tensor_tensor(out=d2[:], in0=bv[:], in1=modes[:], op=ALU.subtract)
        s1 = ptile("s1")
        nc.vector.reciprocal(s1[:], d1[:])
        t1 = ptile("t1")
        nc.vector.reciprocal(t1[:], d2[:])
        clw = ptile("clw")
        nc.vector.tensor_scalar(out=clw[:], in0=d1[:], scalar1=8.0, scalar2=None, op0=ALU.mult)
        crn = ptile("crn")
        nc.vector.tensor_scalar(out=crn[:], in0=d2[:], scalar1=-8.0, scalar2=None, op0=ALU.mult)
        nc.vector.tensor_tensor(out=clw[:, 1:2], in0=clw[:, 1:2], in1=msk255[:], op=ALU.mult)
        nc.vector.tensor_tensor(out=crn[:, 1:2], in0=crn[:, 1:2], in1=msk255[:], op=ALU.mult)
        l0 = ptile("l0")
        nc.vector.tensor_scalar(out=l0[:], in0=av[:], scalar1=float(L - 1), scalar2=None, op0=ALU.mult)
        l0i = smp.tile([P, 2], mybir.dt.int32, tag="l0i", name="l0i")
        nc.vector.tensor_copy(l0i[:], l0[:])
        nc.vector.tensor_scalar(out=l0i[:], in0=l0i[:], scalar1=-32, scalar2=None,
                                op0=ALU.bitwise_and)
        l0q = ptile("l0q")
        nc.vector.tensor_copy(l0q[:], l0i[:])
        qk = ptile("qk")
        nc.vector.tensor_scalar(out=qk[:], in0=l0q[:], scalar1=1.0 / 32.0, scalar2=None, op0=ALU.mult)
        uu = ptile("uu")
        nc.vector.tensor_scalar(out=uu[:], in0=l0q[:], scalar1=1.0 / (L - 1), scalar2=None, op0=ALU.mult)
        bAL = ptile("bAL")
        nc.vector.tensor_tensor(out=bAL[:], in0=uu[:], in1=av[:], op=ALU.subtract)
        nc.vector.tensor_tensor(out=bAL[:], in0=bAL[:], in1=s1[:], op=ALU.mult)
        sAL = ptile("sAL")
        nc.vector.tensor_scalar(out=sAL[:], in0=s1[:], scalar1=1.0 / (L - 1), scalar2=None, op0=ALU.mult)
        bAR = ptile("bAR")
        nc.vector.tensor_tensor(out=bAR[:], in0=bv[:], in1=uu[:], op=ALU.subtract)
        nc.vector.tensor_tensor(out=bAR[:], in0=bAR[:], in1=t1[:], op=ALU.mult)
        sAR = ptile("sAR")
        nc.vector.tensor_scalar(out=sAR[:], in0=t1[:], scalar1=-1.0 / (L - 1), scalar2=None, op0=ALU.mult)

        # ================= window eval + selection matmuls =======================
        pqt = pq.tile([P, 1024], F32, space="PSUM", tag="q")
        mm_i = 0
        for mt in range(2):
            sel = win_pool.tile([P, P], F32, tag="sel", bufs=2)
            nc.vector.tensor_tensor(out=sel[:], in0=qiota[:],
                                    in1=qk[:, mt:mt + 1].to_broadcast([P, P]),
                                    op=ALU.is_equal)
            for side in range(2):  # 0 = L, 1 = R
                sA = sAL if side == 0 else sAR
                bA = bAL if side == 0 else bAR
                wt = win_pool.tile([P, WIN], F32, tag="wt", bufs=2)
                nc.scalar.activation(wt[:], iotaw[:], AF.Relu,
                                     bias=bA[:, mt:mt + 1], scale=sA[:, mt:mt + 1])
                nc.scalar.activation(wt[:], wt[:], AF.Square)
                w4t = win_pool.tile([P, WIN], F32, tag="w4t", bufs=2)
                nc.vector.tensor_tensor(out=w4t[:], in0=wt[:], in1=wt[:], op=ALU.mult)
                nc.vector.tensor_tensor(out=w4t[:], in0=w4t[:], in1=w4t[:], op=ALU.mult)
                nc.vector.tensor_scalar(out=w4t[:], in0=w4t[:], scalar1=1.0, scalar2=None, op0=ALU.min)
                w16 = win_pool.tile([P, WIN], F32, tag="w16", bufs=2)
                nc.vector.tensor_tensor(out=w16[:], in0=w4t[:], in1=w4t[:], op=ALU.mult)
                dd = win_pool.tile([P, WIN], F32, tag="dd", bufs=2)
                if side == 0:
                    nc.vector.tensor_copy(dd[:, 0:1], w16[:, 0:1])
                else:
                    nc.vector.tensor_scalar(out=dd[:, 0:1], in0=w16[:, 0:1],
                                            scalar1=1.0, scalar2=None, op0=ALU.subtract)
                nc.vector.tensor_tensor(out=dd[:, 1:WIN], in0=w16[:, 1:WIN],
                                        in1=w16[:, 0:WIN - 1], op=ALU.subtract)
                selw = win_pool.tile([P, P], F32, tag="selw", bufs=2)
                wvec = clw if side == 0 else crn
                nc.vector.tensor_scalar(out=selw[:], in0=sel[:], scalar1=wvec[:, mt:mt + 1],
                                        scalar2=None, op0=ALU.mult)
                nc.tensor.matmul(pqt[:, 0:512], lhsT=selw[:], rhs=dd[:, 0:512],
                                 start=(mm_i == 0), stop=(mm_i == 3))
                nc.tensor.matmul(pqt[:, 512:WIN], lhsT=selw[:], rhs=dd[:, 512:WIN],
                                 start=(mm_i == 0), stop=(mm_i == 3))
                mm_i += 1
        qsb = win_pool.tile([P, WIN], F32, tag="qsb", bufs=2)
        nc.vector.tensor_copy(qsb[:], pqt[:, 0:WIN])

        # ================= shift-realign + cumsum ================================
        pgb = psm.tile([P, 512], F32, space="PSUM", tag="sm")
        for d in range(ND):
            nc.tensor.matmul(pgb[:, 0:32], lhsT=shiftm[:, ND - d:ND - d + P],
                             rhs=qsb[:, 32 * d:32 * d + 32],
                             start=(d == 0), stop=(d == ND - 1))
        cur = smp.tile([P, 32], F32, tag="cs0")
        nc.vector.tensor_copy(cur[:], pgb[:, 0:32])
        for si, sft in enumerate((1, 2, 4, 8, 16)):
            nxt = smp.tile([P, 32], F32, tag=f"cs{si + 1}", name=f"csn{si}")
            nc.vector.tensor_copy(nxt[:, 0:sft], cur[:, 0:sft])
            nc.vector.tensor_tensor(out=nxt[:, sft:32], in0=cur[:, sft:32],
                                    in1=cur[:, 0:32 - sft], op=ALU.add)
            cur = nxt
        ppf = psm.tile([P, 512], F32, space="PSUM", tag="sm")
        nc.tensor.matmul(ppf[:, 0:1], lhsT=tris[:], rhs=cur[:, 31:32], start=True, stop=True)
        pfx = smp.tile([P, 1], F32, tag="pfx")
        nc.vector.tensor_copy(pfx[:], ppf[:, 0:1])
        gs_t = smp.tile([P, 32], F32, tag="gst")
        nc.vector.tensor_scalar(out=gs_t[:], in0=cur[:], scalar1=pfx[:, 0:1],
                                scalar2=None, op0=ALU.add)
        nc.sync.dma_start(out=gs_d[s].rearrange("(p j) -> p j", p=P), in_=gs_t[:])

        # ======== almat: gpsimd local_scatter of (1-f, f) pairs + cast DMA ======
        kci = smp.tile([P, 32], mybir.dt.int32, tag="kci", name="kci")
        nc.vector.tensor_copy(kci[:], gs_t[:])
        kcf = smp.tile([P, 32], F32, tag="kcf")
        nc.vector.tensor_copy(kcf[:], kci[:])
        gtt = smp.tile([P, 32], F32, tag="gtt")
        nc.vector.tensor_tensor(out=gtt[:], in0=kcf[:], in1=gs_t[:], op=ALU.is_gt)
        k0f = smp.tile([P, 32], F32, tag="k0f")
        nc.vector.tensor_tensor(out=k0f[:], in0=kcf[:], in1=gtt[:], op=ALU.subtract)
        nc.vector.tensor_scalar(out=k0f[:], in0=k0f[:], scalar1=float(K - 2),
                                scalar2=None, op0=ALU.min)
        fr2 = smp.tile([P, 32], F32, tag="fr2")
        nc.vector.tensor_tensor(out=fr2[:], in0=gs_t[:], in1=k0f[:], op=ALU.subtract)
        pairs = smp.tile([P, 32, 2], F32, tag="pairs")
        nc.vector.tensor_copy(pairs[:, :, 1], fr2[:])
        nc.vector.tensor_scalar(out=pairs[:, :, 0], in0=fr2[:], scalar1=-1.0,
                                scalar2=1.0, op0=ALU.mult, op1=ALU.add)
        d16 = smp.tile([P, 32, 2], mybir.dt.float16, tag="d16", name="d16")
        nc.vector.tensor_copy(d16[:], pairs[:])
        idxf = smp.tile([P, 32, 2], F32, tag="idxf")
        nc.vector.tensor_tensor(out=idxf[:, :, 0], in0=k0f[:], in1=lidx[:], op=ALU.add)
        nc.vector.tensor_scalar(out=idxf[:, :, 1], in0=idxf[:, :, 0], scalar1=1.0,
                                scalar2=None, op0=ALU.add)
        i16 = smp.tile([P, 32, 2], mybir.dt.int16, tag="i16", name="i16")
        nc.vector.tensor_copy(i16[:], idxf[:])
        alv = al_d[s].rearrange("(p j) k -> p (j k)", p=P)
        for e in range(8):
            dst16 = win_pool.tile([P, 1024], mybir.dt.float16, tag="dst16", bufs=2)
            nc.gpsimd.local_scatter(out_ap=dst16[:], data_ap=d16[:, 4 * e:4 * e + 4, :],
                                    idxs_ap=i16[:, 4 * e:4 * e + 4, :],
                                    channels=P, num_elems=1024, num_idxs=8)
            dst32 = win_pool.tile([P, 1024], F32, tag="dst32", bufs=3)
            if s == 0:
                nc.gpsimd.tensor_copy(dst32[:], dst16[:])
            else:
                nc.vector.tensor_copy(dst32[:], dst16[:])
            nc.sync.dma_start(out=alv[:, 1024 * e:1024 * (e + 1)], in_=dst32[:])


# ================================ host side ==================================

def _consts():
    c = {}
    c["ident"] = np.eye(P, dtype=np.float32)
    c["tris"] = np.triu(np.ones((P, P), dtype=np.float32), 1)
    c["onesm"] = np.ones((P, P), dtype=np.float32)
    c["shiftm"] = np.eye(P, P + ND, k=ND, dtype=np.float32)
    c["iotaw"] = np.tile(np.arange(WIN, dtype=np.float32), (P, 1))
    c["kiota"] = np.tile(np.arange(K, dtype=np.float32), (P, 1))
    c["qiota"] = np.tile(np.arange(P, dtype=np.float32), (P, 1))
    m = np.ones((P, 1), dtype=np.float32); m[P - 1, 0] = 0.0
    c["msk255"] = m
    h = np.zeros((P, 1), dtype=np.float32); h[P - 1, 0] = 0.5
    c["half255"] = h
    li = np.tile((np.arange(NLC) % 4) * K, (P, 1))
    c["lidx"] = li.astype(np.float32)
    return c


_COMPILED = None
LAST_EXEC_NS = None


def _get_compiled():
    global _COMPILED
    if _COMPILED is None:
        nc = bacc.Bacc("TRN2", target_bir_lowering=False, debug=False,
                       num_devices=NCORES)
        build_graph(nc)
        _COMPILED = nc
    return _COMPILED


def kernel(input_seq, mask, W1, b1, W2, Wr, br, _run_kwargs=None):
    input_seq = np.asarray(input_seq, dtype=np.float32)
    W1 = np.ascontiguousarray(np.asarray(W1, dtype=np.float32))
    W2 = np.asarray(W2, dtype=np.float32)
    Wr = np.ascontiguousarray(np.asarray(Wr, dtype=np.float32))
    B = input_seq.shape[0]
    w2p = np.ascontiguousarray(
        np.concatenate([W2, np.zeros((2 * D, 1), np.float32)], axis=1))
    consts = _consts()
    nc = _get_compiled()
    in_maps = []
    for cix in range(NCORES):
        m = dict(consts)
        m["x"] = np.ascontiguousarray(input_seq[cix * SPC:(cix + 1) * SPC])
        m["w1"] = W1
        m["w2"] = w2p
        m["wr"] = Wr
        in_maps.append(m)
    res = run_bass_kernel_spmd(nc, in_maps, core_ids=list(range(NCORES)),
                               **(_run_kwargs or {}))
    global LAST_EXEC_NS
    if getattr(res, "exec_time_ns", None):
        LAST_EXEC_NS = res.exec_time_ns
    gs = np.concatenate([res.results[i]["gs_out"] for i in range(NCORES)], axis=0)
    al = np.concatenate([res.results[i]["al_out"] for i in range(NCORES)], axis=0)
    return gs.reshape(B, L).astype(np.float32), al.reshape(B, L, K).astype(np.float32)
